# revision 11
# baseline (speedup 1.0000x reference)
"""Causal self-attention (B=2, S=2048, E=1024, H=16, D=64) on 8 TRN2 cores.

Sharding: core c = (batch b = c//4, head-group g = c%4) owns batch b and
heads 4g..4g+3 (a 256-wide slice of the QKV projections / Wo rows).
Each core computes its partial out-projection y_c = attout_c @ Wo_c; the
host sums the 4 partials per batch and adds bo (the tensor-parallel
out-proj all-reduce, done on host since cores are independent).

All device inputs/outputs are host-packed into [128, X] layouts whose
per-partition rows are contiguous in DRAM, so every DMA is 128 large
contiguous descriptors (DMA-issue cost on the sync sequencer would
otherwise dominate the kernel).

Device pipeline (per core), bf16 operands / fp32 PSUM accumulation:
  Q^T, K^T [256, S] via lhsT=W chunk, rhs=xT chunk
  V natural [S, 4*(64+1)] with a ones column per head (softmax denom)
  scores^T [k, q] per head: lhsT=K^T (D=64 contraction, head pairs
  packed in PE row-groups 0-63/64-127), exp on ACT (scale=1/8 folded),
  causal mask multiply on DVE (diagonal blocks only; upper blocks
  skipped entirely)
  attout^T [65, q] PV matmul, row 64 = softmax denominator
  normalize: reciprocal_approx_fast + gpsimd partition_broadcast + TT mul
  y = attoutT_norm.T @ Wo_c, staged in SBUF, DMA'd out in 4 chunks.
"""

import numpy as np

B, S, E, H = 2, 2048, 1024, 16
D = E // H          # 64
NCORES = 8
HPC = 4             # heads per core
HD = HPC * D        # 256 cols per core
KT = E // 128       # 8 contraction tiles for projections
QC = S // 512       # 4 query chunks
NQT = S // 128      # 16 row tiles
VW = HPC * (D + 1)  # 260: V + ones column per head

_prog = None
LAST_RESULTS = None


def _build_program():
    import concourse.mybir as mybir
    import concourse.tile as tile
    from concourse import bacc, library_config

    f32 = mybir.dt.float32
    bf16 = mybir.dt.bfloat16
    Exp = mybir.ActivationFunctionType.Exp

    nc = bacc.Bacc(trn_type="TRN2", target_bir_lowering=False, debug=False)

    xT = nc.dram_tensor("xT", [128, QC * KT * 512], bf16, kind="ExternalInput").ap()
    wq = nc.dram_tensor("wq", [128, KT * HD], bf16, kind="ExternalInput").ap()
    wk = nc.dram_tensor("wk", [128, KT * HD], bf16, kind="ExternalInput").ap()
    wv = nc.dram_tensor("wv", [128, KT * HD], bf16, kind="ExternalInput").ap()
    wo = nc.dram_tensor("wo", [128, 2 * E], bf16, kind="ExternalInput").ap()
    bq = nc.dram_tensor("bq", [1, HD], bf16, kind="ExternalInput").ap()
    bk = nc.dram_tensor("bk", [1, HD], bf16, kind="ExternalInput").ap()
    bv = nc.dram_tensor("bv", [1, HD], bf16, kind="ExternalInput").ap()
    mask = nc.dram_tensor("mask", [128, 4 * 512], bf16, kind="ExternalInput").ap()
    y = nc.dram_tensor("y", [128, NQT * E], f32, kind="ExternalOutput").ap()

    with tile.TileContext(nc) as tc:
        with (
            tc.tile_pool(name="consts", bufs=1) as consts,
            tc.tile_pool(name="exps", bufs=4) as exps,
            tc.tile_pool(name="small", bufs=4) as small,
            tc.tile_pool(name="ps_sc", bufs=3, space="PSUM") as ps_sc,
            tc.tile_pool(name="ps_acc", bufs=2, space="PSUM") as ps_acc,
        ):
            nc.gpsimd.load_library(library_config.attn)
            # ---- constants (one big contiguous DMA each) ----
            xt_sb = consts.tile([128, QC, KT, 512], bf16)
            for qc in range(QC):
                nc.sync.dma_start(
                    out=xt_sb[:, qc],
                    in_=xT[:, qc * KT * 512 : (qc + 1) * KT * 512].rearrange(
                        "p (kt c) -> p kt c", kt=KT
                    ),
                )
            wq_sb = consts.tile([128, KT, HD], bf16)
            nc.sync.dma_start(out=wq_sb, in_=wq.rearrange("p (kt c) -> p kt c", kt=KT))
            wk_sb = consts.tile([128, KT, HD], bf16)
            nc.sync.dma_start(out=wk_sb, in_=wk.rearrange("p (kt c) -> p kt c", kt=KT))
            wv_sb = consts.tile([128, KT, HD], bf16)
            nc.sync.dma_start(out=wv_sb, in_=wv.rearrange("p (kt c) -> p kt c", kt=KT))
            wo_sb = consts.tile([128, 2, E], bf16)
            nc.sync.dma_start(out=wo_sb, in_=wo.rearrange("p (kt c) -> p kt c", kt=2))
            mask_sb = consts.tile([128, 4, 512], bf16)
            nc.sync.dma_start(out=mask_sb, in_=mask.rearrange("p (t c) -> p t c", t=4))
            bq_sb = consts.tile([1, HD], bf16)
            nc.sync.dma_start(out=bq_sb, in_=bq)
            bk_sb = consts.tile([1, HD], bf16)
            nc.sync.dma_start(out=bk_sb, in_=bk)
            bv_sb = consts.tile([1, HD], bf16)
            nc.sync.dma_start(out=bv_sb, in_=bv)
            ones_sb = consts.tile([1, 512], bf16)
            nc.vector.memset(ones_sb, 1.0)

            # ---- persistent activations ----
            # Q^T/K^T: [128, mt, S]; mt=0 holds cols 0-127 (heads 0,1),
            # mt=1 holds cols 128-255 (heads 2,3).
            qt_sb = consts.tile([128, 2, S], bf16)
            kt_sb = consts.tile([128, 2, S], bf16)
            # V natural: [row-in-tile, rt, 4*(64+1)]; per head h cols
            # h*65..h*65+63 are V, col h*65+64 is ones.
            v_sb = consts.tile([128, NQT, VW], bf16)
            nc.vector.memset(
                v_sb.rearrange("p rt (h c) -> p rt h c", h=HPC)[:, :, :, D : D + 1],
                1.0,
            )
            # normalized attout^T, same layout as qt_sb
            at_sb = consts.tile([128, 2, S], bf16)
            # full output staging: [p, qt, col]
            y_sb = consts.tile([128, NQT, E], f32)

            # ================= phase 1: projections =================
            for qc in range(QC):
                for w_sb, b_sb, dst in ((wq_sb, bq_sb, qt_sb), (wk_sb, bk_sb, kt_sb)):
                    ps = ps_sc.tile([128, 1024], f32, tag="sc", name=f"ps_qk{qc}")
                    for mt in range(2):
                        o = ps[:, mt * 512 : mt * 512 + 512]
                        for kt in range(KT):
                            nc.tensor.matmul(
                                o,
                                lhsT=w_sb[:, kt, mt * 128 : mt * 128 + 128],
                                rhs=xt_sb[:, qc, kt],
                                start=(kt == 0),
                                stop=False,
                            )
                        nc.tensor.matmul(
                            o,
                            lhsT=b_sb[:, mt * 128 : mt * 128 + 128],
                            rhs=ones_sb,
                            start=False,
                            stop=True,
                        )
                    nc.vector.tensor_copy(
                        dst[:, :, qc * 512 : (qc + 1) * 512],
                        ps.rearrange("p (mt q) -> p mt q", mt=2),
                    )

                for half in range(2):  # two V psum tiles, 2 row-tiles each
                    ps = ps_sc.tile([128, 1024], f32, tag="sc", name=f"ps_v{qc}_{half}")
                    for j in range(2):
                        rl = half * 2 + j          # row-tile within chunk (0..3)
                        rt = qc * 4 + rl           # global row tile
                        o = ps[:, j * 512 : j * 512 + HD]
                        for kt in range(KT):
                            nc.tensor.matmul(
                                o,
                                lhsT=xt_sb[:, qc, kt, rl * 128 : rl * 128 + 128],
                                rhs=wv_sb[:, kt, :],
                                start=(kt == 0),
                                stop=False,
                            )
                        nc.tensor.matmul(
                            o,
                            lhsT=ones_sb[:, 0:128],
                            rhs=bv_sb,
                            start=False,
                            stop=True,
                        )
                        nc.vector.tensor_copy(
                            v_sb[:, rt, :]
                            .rearrange("p (h c) -> p h c", h=HPC)[:, :, 0:D],
                            o.rearrange("p (h c) -> p h c", h=HPC),
                        )

            # ================= phase 2: attention =================
            for mt in range(2):              # head pair (2mt, 2mt+1)
                for qc in range(QC):
                    nkt = 4 * (qc + 1)       # causal: k-tiles 0..nkt-1
                    acc = [
                        ps_acc.tile([128, 512], f32, tag="acc", name=f"acc{mt}{qc}{j}")
                        for j in range(2)
                    ]
                    for kt in range(nkt):
                        ps = ps_sc.tile([128, 1024], f32, tag="sc", name=f"ps_s{kt}")
                        for j in range(2):   # head within pair
                            pb = j * 64
                            nc.tensor.matmul(
                                ps[:, j * 512 : j * 512 + 512],
                                lhsT=kt_sb[pb : pb + 64, mt, kt * 128 : kt * 128 + 128],
                                rhs=qt_sb[pb : pb + 64, mt, qc * 512 : qc * 512 + 512],
                                start=True,
                                stop=True,
                            )
                        ex = exps.tile([128, 1024], bf16, tag="ex", name=f"ex{kt}")
                        # scores scale 1/sqrt(D) folded into exp
                        nc.scalar.activation(ex, ps, Exp, scale=0.125)
                        t = kt - 4 * qc
                        for j in range(2):
                            exj = ex[:, j * 512 : j * 512 + 512]
                            if t >= 0:  # diagonal block: causal mask
                                nc.vector.tensor_mul(exj, exj, mask_sb[:, t, :])
                            h = 2 * mt + j
                            nc.tensor.matmul(
                                acc[j][0:65, :],
                                lhsT=v_sb[:, kt, h * 65 : h * 65 + 65],
                                rhs=exj,
                                start=(kt == 0),
                                stop=(kt == nkt - 1),
                            )
                    for j in range(2):
                        dn = small.tile([1, 512], f32, tag="dn", name=f"dn{j}")
                        # reciprocal_approx_fast misreads PSUM on HW; bounce
                        # the denominator row through SBUF first.
                        nc.vector.tensor_copy(dn, acc[j][64:65, :])
                        rc = small.tile([1, 512], f32, tag="rc", name=f"rc{j}")
                        nc.vector.reciprocal_approx_fast(out=rc, in_=dn)
                        bc = small.tile([64, 512], f32, tag="bc", name=f"bc{j}")
                        nc.gpsimd.partition_broadcast(out_ap=bc, in_ap=rc)
                        pb = j * 64
                        nc.vector.tensor_mul(
                            at_sb[pb : pb + 64, mt, qc * 512 : qc * 512 + 512],
                            acc[j][0:64, :],
                            bc,
                        )

            # ================= phase 3: out projection =================
            for qg in range(4):              # quarters, DMA'd out eagerly
                for qt in range(qg * 4, qg * 4 + 4):
                    for nh in range(2):
                        ps = ps_acc.tile(
                            [128, 512], f32, tag="acc", name=f"ps_y{qt}{nh}"
                        )
                        for kt2 in range(2):
                            nc.tensor.matmul(
                                ps,
                                lhsT=at_sb[:, kt2, qt * 128 : qt * 128 + 128],
                                rhs=wo_sb[:, kt2, nh * 512 : nh * 512 + 512],
                                start=(kt2 == 0),
                                stop=(kt2 == 1),
                            )
                        nc.vector.tensor_copy(
                            y_sb[:, qt, nh * 512 : nh * 512 + 512], ps
                        )
                nc.sync.dma_start(
                    out=y[:, qg * 4 * E : (qg + 1) * 4 * E],
                    in_=y_sb[:, qg * 4 : (qg + 1) * 4, :],
                )

    nc.compile()
    return nc


def _get_program():
    global _prog
    if _prog is None:
        _prog = _build_program()
    return _prog


def _make_mask():
    import ml_dtypes

    k = np.arange(128)[:, None]
    q = np.arange(512)[None, :]
    m = np.stack([(q >= k + 128 * t) for t in range(4)])  # [4, 128, 512]
    return np.ascontiguousarray(
        m.transpose(1, 0, 2).reshape(128, 4 * 512)
    ).astype(ml_dtypes.bfloat16)


def _pack_rows(a, ktiles):
    """[ktiles*128, C] -> [128, ktiles*C] with per-partition contiguous rows."""
    kt, c = ktiles, a.shape[1]
    return np.ascontiguousarray(
        a.reshape(kt, 128, c).transpose(1, 0, 2).reshape(128, kt * c)
    )


def _core_inputs(x, Wq, bq, Wk, bk, Wv, bv, Wo, mask, c):
    import ml_dtypes

    bf16 = ml_dtypes.bfloat16
    b, g = divmod(c, 4)
    sl = slice(g * HD, (g + 1) * HD)
    xT = x[b].T  # [E, S]
    xT_p = np.ascontiguousarray(
        xT.reshape(KT, 128, QC, 512).transpose(1, 2, 0, 3).reshape(128, QC * KT * 512)
    )
    return {
        "xT": xT_p.astype(bf16),
        "wq": _pack_rows(Wq[:, sl], KT).astype(bf16),
        "wk": _pack_rows(Wk[:, sl], KT).astype(bf16),
        "wv": _pack_rows(Wv[:, sl], KT).astype(bf16),
        "wo": _pack_rows(Wo[sl, :], 2).astype(bf16),
        "bq": np.ascontiguousarray(bq[sl])[None, :].astype(bf16),
        "bk": np.ascontiguousarray(bk[sl])[None, :].astype(bf16),
        "bv": np.ascontiguousarray(bv[sl])[None, :].astype(bf16),
        "mask": mask,
    }


def _unpack_y(y_p):
    """[128, NQT*E] -> [S, E]"""
    return y_p.reshape(128, NQT, E).transpose(1, 0, 2).reshape(S, E)


def kernel(x, Wq, bq, Wk, bk, Wv, bv, Wo, bo, **_run_kwargs):
    from concourse.bass_utils import run_bass_kernel_spmd

    x = np.asarray(x, dtype=np.float32)
    Wq, bq = np.asarray(Wq, np.float32), np.asarray(bq, np.float32)
    Wk, bk = np.asarray(Wk, np.float32), np.asarray(bk, np.float32)
    Wv, bv = np.asarray(Wv, np.float32), np.asarray(bv, np.float32)
    Wo, bo = np.asarray(Wo, np.float32), np.asarray(bo, np.float32)

    nc = _get_program()
    mask = _make_mask()
    in_maps = [
        _core_inputs(x, Wq, bq, Wk, bk, Wv, bv, Wo, mask, c) for c in range(NCORES)
    ]
    res = run_bass_kernel_spmd(nc, in_maps, list(range(NCORES)), **_run_kwargs)
    global LAST_RESULTS
    LAST_RESULTS = res
    parts = [_unpack_y(res.results[c]["y"]) for c in range(NCORES)]
    out = np.empty((B, S, E), np.float32)
    for b in range(B):
        out[b] = parts[4 * b] + parts[4 * b + 1] + parts[4 * b + 2] + parts[4 * b + 3]
        out[b] += bo
    return out


# revision 17
# speedup vs baseline: 1.8645x; 1.8645x over previous
"""Causal self-attention (B=2, S=2048, E=1024, H=16, D=64) on 8 TRN2 cores.

Sharding: core c = (batch b = c//4, head-group g = c%4) owns batch b and
heads 4g..4g+3 (a 256-wide slice of the QKV projections / Wo rows).
Each core computes its partial out-projection y_c = attout_c @ Wo_c; the
host sums the 4 partials per batch and adds bo (the tensor-parallel
out-proj all-reduce, done on host since cores are independent).

All device inputs/outputs are host-packed into [128, X] layouts whose
per-partition rows are contiguous in DRAM, so every DMA is 128 large
contiguous descriptors (DMA-issue cost on the sync sequencer would
otherwise dominate the kernel).

Device pipeline (per core), bf16 operands / fp32 PSUM accumulation:
  Q^T, K^T [256, S] via lhsT=W chunk, rhs=xT chunk
  V natural [S, 4*(64+1)] with a ones column per head (softmax denom)
  scores^T [k, q] per head: lhsT=K^T (D=64 contraction, head pairs
  packed in PE row-groups 0-63/64-127), exp on ACT (scale=1/8 folded),
  causal mask multiply on DVE (diagonal blocks only; upper blocks
  skipped entirely)
  attout^T [65, q] PV matmul, row 64 = softmax denominator
  normalize: reciprocal_approx_fast + gpsimd partition_broadcast + TT mul
  y = attoutT_norm.T @ Wo_c, staged in SBUF, DMA'd out in 4 chunks.
"""

import numpy as np

B, S, E, H = 2, 2048, 1024, 16
D = E // H          # 64
NCORES = 8
HPC = 4             # heads per core
HD = HPC * D        # 256 cols per core
KT = E // 128       # 8 contraction tiles for projections
QC = S // 512       # 4 query chunks
NQT = S // 128      # 16 row tiles
VW = HPC * (D + 1)  # 260: V + ones column per head

_prog = None
LAST_RESULTS = None


def _build_program():
    import concourse.mybir as mybir
    import concourse.tile as tile
    from concourse import bacc, library_config

    f32 = mybir.dt.float32
    bf16 = mybir.dt.bfloat16
    Exp = mybir.ActivationFunctionType.Exp
    Identity = mybir.ActivationFunctionType.Identity

    nc = bacc.Bacc(trn_type="TRN2", target_bir_lowering=False, debug=False)

    xT = nc.dram_tensor("xT", [128, QC * KT * 512], bf16, kind="ExternalInput").ap()
    wq = nc.dram_tensor("wq", [128, KT * HD], bf16, kind="ExternalInput").ap()
    wk = nc.dram_tensor("wk", [128, KT * HD], bf16, kind="ExternalInput").ap()
    wv = nc.dram_tensor("wv", [128, KT * HD], bf16, kind="ExternalInput").ap()
    wo = nc.dram_tensor("wo", [128, 2 * E], bf16, kind="ExternalInput").ap()
    bq = nc.dram_tensor("bqc", [128, 2], f32, kind="ExternalInput").ap()
    bk = nc.dram_tensor("bkc", [128, 2], f32, kind="ExternalInput").ap()
    bv = nc.dram_tensor("bvb", [128, HD], bf16, kind="ExternalInput").ap()
    mask = nc.dram_tensor("mask", [128, 4 * 512], bf16, kind="ExternalInput").ap()
    y = nc.dram_tensor("y", [128, NQT * E], f32, kind="ExternalOutput").ap()

    with tile.TileContext(nc) as tc:
        with (
            tc.tile_pool(name="consts", bufs=1) as consts,
            tc.tile_pool(name="exps", bufs=4) as exps,
            tc.tile_pool(name="small", bufs=4) as small,
            tc.tile_pool(name="ps_sc", bufs=3, space="PSUM") as ps_sc,
            tc.tile_pool(name="ps_acc", bufs=2, space="PSUM") as ps_acc,
        ):
            nc.gpsimd.load_library(library_config.attn)
            # ---- constants; DMA order tuned so qc=0 work starts ASAP ----
            xt_sb = consts.tile([128, QC, KT, 512], bf16)
            wq_sb = consts.tile([128, KT, HD], bf16)
            wk_sb = consts.tile([128, KT, HD], bf16)
            wv_sb = consts.tile([128, KT, HD], bf16)
            wo_sb = consts.tile([128, 2, E], bf16)
            mask_sb = consts.tile([128, 4, 512], bf16)
            bq_sb = consts.tile([128, 2], f32)
            bk_sb = consts.tile([128, 2], f32)
            bv_sb = consts.tile([128, HD], bf16)

            def load_xt(qc):
                nc.sync.dma_start(
                    out=xt_sb[:, qc],
                    in_=xT[:, qc * KT * 512 : (qc + 1) * KT * 512].rearrange(
                        "p (kt c) -> p kt c", kt=KT
                    ),
                )

            nc.sync.dma_start(out=wq_sb, in_=wq.rearrange("p (kt c) -> p kt c", kt=KT))
            load_xt(0)
            nc.sync.dma_start(out=wk_sb, in_=wk.rearrange("p (kt c) -> p kt c", kt=KT))
            nc.sync.dma_start(out=wv_sb, in_=wv.rearrange("p (kt c) -> p kt c", kt=KT))
            nc.sync.dma_start(out=bq_sb, in_=bq)
            nc.sync.dma_start(out=bk_sb, in_=bk)
            nc.sync.dma_start(out=bv_sb, in_=bv)
            load_xt(1)
            nc.sync.dma_start(out=mask_sb, in_=mask.rearrange("p (t c) -> p t c", t=4))
            load_xt(2)
            load_xt(3)
            nc.sync.dma_start(out=wo_sb, in_=wo.rearrange("p (kt c) -> p kt c", kt=2))

            # ---- persistent activations ----
            # Q^T/K^T: [128, mt, S]; mt=0 holds cols 0-127 (heads 0,1),
            # mt=1 holds cols 128-255 (heads 2,3).
            qt_sb = consts.tile([128, 2, S], bf16)
            kt_sb = consts.tile([128, 2, S], bf16)
            # V natural: [row-in-tile, rt, 4*(64+1)]; per head h cols
            # h*65..h*65+63 are V, col h*65+64 is ones.
            v_sb = consts.tile([128, NQT, VW], bf16)
            nc.vector.memset(
                v_sb.rearrange("p rt (h c) -> p rt h c", h=HPC)[:, :, :, D : D + 1],
                1.0,
            )
            # normalized attout^T, same layout as qt_sb
            at_sb = consts.tile([128, 2, S], bf16)
            # full output staging: [p, qt, col]
            y_sb = consts.tile([128, NQT, E], f32)

            # ================= phase 1: projections =================
            for qc in range(QC):
                for w_sb, b_sb, dst in ((wq_sb, bq_sb, qt_sb), (wk_sb, bk_sb, kt_sb)):
                    ps = ps_sc.tile([128, 1024], f32, tag="sc", name=f"ps_qk{qc}")
                    for mt in range(2):
                        o = ps[:, mt * 512 : mt * 512 + 512]
                        for kt in range(KT):
                            nc.tensor.matmul(
                                o,
                                lhsT=w_sb[:, kt, mt * 128 : mt * 128 + 128],
                                rhs=xt_sb[:, qc, kt],
                                start=(kt == 0),
                                stop=(kt == KT - 1),
                            )
                        # PSUM->SBUF copy on ACT with the bias folded in
                        nc.scalar.activation(
                            dst[:, mt, qc * 512 : (qc + 1) * 512],
                            o,
                            Identity,
                            bias=b_sb[:, mt : mt + 1],
                        )

                for half in range(2):  # two V psum tiles, 2 row-tiles each
                    ps = ps_sc.tile([128, 1024], f32, tag="sc", name=f"ps_v{qc}_{half}")
                    for j in range(2):
                        rl = half * 2 + j          # row-tile within chunk (0..3)
                        rt = qc * 4 + rl           # global row tile
                        o = ps[:, j * 512 : j * 512 + HD]
                        for kt in range(KT):
                            nc.tensor.matmul(
                                o,
                                lhsT=xt_sb[:, qc, kt, rl * 128 : rl * 128 + 128],
                                rhs=wv_sb[:, kt, :],
                                start=(kt == 0),
                                stop=(kt == KT - 1),
                            )
                        # PSUM->SBUF with bias added (bvb host-broadcast)
                        nc.vector.tensor_add(
                            v_sb[:, rt, :]
                            .rearrange("p (h c) -> p h c", h=HPC)[:, :, 0:D],
                            o.rearrange("p (h c) -> p h c", h=HPC),
                            bv_sb.rearrange("p (h c) -> p h c", h=HPC),
                        )

            # ================= phase 2: attention =================
            for mt in range(2):              # head pair (2mt, 2mt+1)
                for qc in range(QC):
                    nkt = 4 * (qc + 1)       # causal: k-tiles 0..nkt-1
                    acc = [
                        ps_acc.tile([128, 512], f32, tag="acc", name=f"acc{mt}{qc}{j}")
                        for j in range(2)
                    ]
                    for kt in range(nkt):
                        ps = ps_sc.tile([128, 1024], f32, tag="sc", name=f"ps_s{kt}")
                        for j in range(2):   # head within pair
                            pb = j * 64
                            nc.tensor.matmul(
                                ps[:, j * 512 : j * 512 + 512],
                                lhsT=kt_sb[pb : pb + 64, mt, kt * 128 : kt * 128 + 128],
                                rhs=qt_sb[pb : pb + 64, mt, qc * 512 : qc * 512 + 512],
                                start=True,
                                stop=True,
                            )
                        ex = exps.tile([128, 1024], bf16, tag="ex", name=f"ex{kt}")
                        # scores scale 1/sqrt(D) folded into exp
                        nc.scalar.activation(ex, ps, Exp, scale=0.125)
                        t = kt - 4 * qc
                        for j in range(2):
                            exj = ex[:, j * 512 : j * 512 + 512]
                            if t >= 0:  # diagonal block: causal mask
                                nc.vector.tensor_mul(exj, exj, mask_sb[:, t, :])
                            h = 2 * mt + j
                            nc.tensor.matmul(
                                acc[j][0:65, :],
                                lhsT=v_sb[:, kt, h * 65 : h * 65 + 65],
                                rhs=exj,
                                start=(kt == 0),
                                stop=(kt == nkt - 1),
                            )
                    for j in range(2):
                        dn = small.tile([1, 512], f32, tag="dn", name=f"dn{j}")
                        # reciprocal_approx_fast misreads PSUM on HW; bounce
                        # the denominator row through SBUF first.
                        nc.vector.tensor_copy(dn, acc[j][64:65, :])
                        rc = small.tile([1, 512], f32, tag="rc", name=f"rc{j}")
                        nc.vector.reciprocal_approx_fast(out=rc, in_=dn)
                        bc = small.tile([64, 512], f32, tag="bc", name=f"bc{j}")
                        nc.gpsimd.partition_broadcast(out_ap=bc, in_ap=rc)
                        pb = j * 64
                        nc.vector.tensor_mul(
                            at_sb[pb : pb + 64, mt, qc * 512 : qc * 512 + 512],
                            acc[j][0:64, :],
                            bc,
                        )

            # ================= phase 3: out projection =================
            for qg in range(4):              # quarters, DMA'd out eagerly
                for qt in range(qg * 4, qg * 4 + 4):
                    for nh in range(2):
                        ps = ps_acc.tile(
                            [128, 512], f32, tag="acc", name=f"ps_y{qt}{nh}"
                        )
                        for kt2 in range(2):
                            nc.tensor.matmul(
                                ps,
                                lhsT=at_sb[:, kt2, qt * 128 : qt * 128 + 128],
                                rhs=wo_sb[:, kt2, nh * 512 : nh * 512 + 512],
                                start=(kt2 == 0),
                                stop=(kt2 == 1),
                            )
                        nc.vector.tensor_copy(
                            y_sb[:, qt, nh * 512 : nh * 512 + 512], ps
                        )
                nc.sync.dma_start(
                    out=y[:, qg * 4 * E : (qg + 1) * 4 * E],
                    in_=y_sb[:, qg * 4 : (qg + 1) * 4, :],
                )

    nc.compile()
    return nc


def _get_program():
    global _prog
    if _prog is None:
        _prog = _build_program()
    return _prog


def _make_mask():
    import ml_dtypes

    k = np.arange(128)[:, None]
    q = np.arange(512)[None, :]
    m = np.stack([(q >= k + 128 * t) for t in range(4)])  # [4, 128, 512]
    return np.ascontiguousarray(
        m.transpose(1, 0, 2).reshape(128, 4 * 512)
    ).astype(ml_dtypes.bfloat16)


def _pack_rows(a, ktiles):
    """[ktiles*128, C] -> [128, ktiles*C] with per-partition contiguous rows."""
    kt, c = ktiles, a.shape[1]
    return np.ascontiguousarray(
        a.reshape(kt, 128, c).transpose(1, 0, 2).reshape(128, kt * c)
    )


def _core_inputs(x, Wq, bq, Wk, bk, Wv, bv, Wo, mask, c):
    import ml_dtypes

    bf16 = ml_dtypes.bfloat16
    b, g = divmod(c, 4)
    sl = slice(g * HD, (g + 1) * HD)
    xT = x[b].T  # [E, S]
    xT_p = np.ascontiguousarray(
        xT.reshape(KT, 128, QC, 512).transpose(1, 2, 0, 3).reshape(128, QC * KT * 512)
    )
    return {
        "xT": xT_p.astype(bf16),
        "wq": _pack_rows(Wq[:, sl], KT).astype(bf16),
        "wk": _pack_rows(Wk[:, sl], KT).astype(bf16),
        "wv": _pack_rows(Wv[:, sl], KT).astype(bf16),
        "wo": _pack_rows(Wo[sl, :], 2).astype(bf16),
        "bqc": np.ascontiguousarray(bq[sl].reshape(2, 128).T).astype(np.float32),
        "bkc": np.ascontiguousarray(bk[sl].reshape(2, 128).T).astype(np.float32),
        "bvb": np.ascontiguousarray(
            np.broadcast_to(bv[sl], (128, HD))
        ).astype(bf16),
        "mask": mask,
    }


def _unpack_y(y_p):
    """[128, NQT*E] -> [S, E]"""
    return y_p.reshape(128, NQT, E).transpose(1, 0, 2).reshape(S, E)


def kernel(x, Wq, bq, Wk, bk, Wv, bv, Wo, bo, **_run_kwargs):
    from concourse.bass_utils import run_bass_kernel_spmd

    x = np.asarray(x, dtype=np.float32)
    Wq, bq = np.asarray(Wq, np.float32), np.asarray(bq, np.float32)
    Wk, bk = np.asarray(Wk, np.float32), np.asarray(bk, np.float32)
    Wv, bv = np.asarray(Wv, np.float32), np.asarray(bv, np.float32)
    Wo, bo = np.asarray(Wo, np.float32), np.asarray(bo, np.float32)

    nc = _get_program()
    mask = _make_mask()
    in_maps = [
        _core_inputs(x, Wq, bq, Wk, bk, Wv, bv, Wo, mask, c) for c in range(NCORES)
    ]
    res = run_bass_kernel_spmd(nc, in_maps, list(range(NCORES)), **_run_kwargs)
    global LAST_RESULTS
    LAST_RESULTS = res
    parts = [_unpack_y(res.results[c]["y"]) for c in range(NCORES)]
    out = np.empty((B, S, E), np.float32)
    for b in range(B):
        out[b] = parts[4 * b] + parts[4 * b + 1] + parts[4 * b + 2] + parts[4 * b + 3]
        out[b] += bo
    return out
